# revision 57
# baseline (speedup 1.0000x reference)
"""Trainium2 Bass kernel for nn_BSAM_60129542251.

Conv-QKV self-attention block (B=4, C=64, H=W=64):
  Q = conv3x3(A1_B, w1)  -> [b, 32, 4096]
  K = conv3x3(A1_C, w2)  -> [b, 32, 4096]
  V = conv3x3(A1_C, w3)  -> [b, 64, 4096]
  E = softmax(Q^T K) V^T -> [b, 4096, 64];  out = E^T + A1_C

Sharding: 8 cores; core i handles sample b=i//2, row-half i%2 (2048 query
rows). K/V convs are duplicated within a sample pair; Q conv runs on the
core's half only. Attention is fully fused on-chip (no S matrix in HBM).

v3 structure (engine-balanced, conv/attention interleaved, software
pipelined):
  - Conv = 9 shifted matmuls done as 6 (dy-pairs stacked in contraction;
    rows 64:128 of the flat input hold a 64-shifted copy), with compact
    edge-correction matmuls cancelling the w=0/w=63 row-wrap reads.
  - Conv epilogues (PSUM->SBUF + bias) run on Pool (K, Q as fp32 bits into
    f32r tiles) and DVE (V -> bf16), keeping ACT free for exp.
  - V tiles transposed via one batched XBAR DMA transpose per conv tile
    ([64,512] -> [128, 4, 64] 3D out) straight into the [128, 65]-per-chunk
    V' stationary layout (col 64 = ones for the softmax denominator row).
  - Attention per (mt, pair-of-2-chunks): 2 QK matmuls (f32r, k chunk
    [32,128] stationary) into a [128,1024] PSUM tile, one exp -> bf16,
    2 PV matmuls (bf16) accumulating E'^T in PSUM ([65, 512], row 64 =
    denominators). exp ops are split between ACT (exact, scale=1/EXPA)
    and DVE/Pool (2^x int16-bitcast approx; EXPA folded into w1).
  - mt0's attention interleaves with the KV conv, QK one conv tile behind,
    PV two behind (hides the K-epilogue and V-transpose latency). mt1..3
    run after with a QK/PV two-stage pipeline. PSUM: 2 conv + 4 st + 2 et.
  - Normalize tail split in column halves across DVE (recip, mul) and
    Pool (broadcast, add) to shrink the exposed critical path at the end.
  - Inputs ride 2 bundled small-weight DMAs + per-half chunked xb/xc
    (sync HWDGE queue for rows 0:64, gpsimd SWDGE for rows 64:128).
"""

import numpy as np

import concourse.bacc as bacc
import concourse.mybir as mybir
import concourse.tile as tile
from concourse import bass_utils
from concourse.masks import make_identity

F32 = mybir.dt.float32
F32R = mybir.dt.float32r
BF16 = mybir.dt.bfloat16
I16 = mybir.dt.int16
AF = mybir.ActivationFunctionType

B, C, CH, H, W = 4, 64, 32, 64, 64
N = H * W                     # 4096 keys
M = N // 2                    # 2048 query rows per core
NCORES = 8
XC_LEN = 4352                 # padded flat A1_C: 66*64+2 = 4226, padded up
XB_LEN = 2304                 # padded flat A1_B half: 34*64+2 = 2178, padded up
NKC = N // 128                # 32 key chunks
NPAIR = NKC // 2              # 16 chunk pairs (one [128,1024] st tile each)

# bund64 column offsets: wc1 | wc23 | ecl | ecr | ebl | ebr
B64_WC1, B64_WC23, B64_ECL, B64_ECR, B64_EBL, B64_EBR, B64_LEN = (
    0, 192, 960, 1026, 1093, 1127, 1162)
# bund128 column offsets: w1t | w23t | b23 | b1 | bv (V bias at rows 0:64)
B128_W1, B128_W23, B128_B23, B128_B1, B128_BV, B128_LEN = 0, 192, 960, 961, 962, 963

# 2^x trick constants: S_psum = EXPA * s (EXPA folded into w1 on host);
# p = exp(s) ~ bitcast_bf16(int16(S_psum + EXPB)).
EXPA = 128.0 / float(np.log(2.0))          # 184.664965...
EXPB_ROUND = 16256.0 - 5.5053              # round-to-nearest int16 convert (HW)
EXPB_TRUNC = EXPB_ROUND + 0.5              # truncate-toward-zero (interp only)


def _exp_engine(mt, p):
    """exp engine for (mt, pair): ACT exact, or DVE 2^x approx."""
    if p % 3 == 2:
        return "dve"
    return "act"


_cache = {}


def _r32r(x):
    """Round fp32 -> float32r (zero low 12 mantissa bits, round to nearest)."""
    x = np.ascontiguousarray(x, np.float32)
    b = x.view(np.uint32).astype(np.uint64)
    out = (((b + np.uint64(1 << 11)) & np.uint64(0xFFFFF000)).astype(np.uint32)).view(np.float32)
    return np.ascontiguousarray(out)


def _build(dbg=False, expb=EXPB_ROUND):
    nc = bacc.Bacc("TRN2", target_bir_lowering=False, debug=False)

    xc = nc.dram_tensor("xc", [128, XC_LEN], F32R, kind="ExternalInput")
    xb = nc.dram_tensor("xb", [128, XB_LEN], F32R, kind="ExternalInput")
    bund64 = nc.dram_tensor("bund64", [C, B64_LEN], BF16, kind="ExternalInput")
    bund128 = nc.dram_tensor("bund128", [128, B128_LEN], F32R, kind="ExternalInput")
    resid = nc.dram_tensor("resid", [C, M], F32, kind="ExternalInput")
    out_d = nc.dram_tensor("out", [C, M], F32, kind="ExternalOutput")
    if dbg:
        k_d = nc.dram_tensor("k_dbg", [CH, N], F32, kind="ExternalOutput")
        q_d = nc.dram_tensor("q_dbg", [CH, M], F32, kind="ExternalOutput")
        v_d = nc.dram_tensor("v_dbg", [128, NKC * 65], F32, kind="ExternalOutput")

    with tile.TileContext(nc) as tc:
        with (
            tc.tile_pool(name="big", bufs=1) as big,
            tc.tile_pool(name="work", bufs=3) as work,
            tc.tile_pool(name="expool", bufs=8) as expool,
        ):
            xc_sb = big.tile([128, XC_LEN], F32R, tag="xc")
            xb_sb = big.tile([128, XB_LEN], F32R, tag="xb")
            b64_sb = big.tile([C, B64_LEN], BF16, tag="b64")
            b128_sb = big.tile([128, B128_LEN], F32R, tag="b128")
            res_sb = big.tile([C, M], F32, tag="res")
            corrS = big.tile([C, 320], F32, tag="corrS")
            k_sb = big.tile([CH, N], F32R, tag="k")
            qt_sb = big.tile([CH, M], F32R, tag="qt")
            v_sb = big.tile([128, NKC * 65], BF16, tag="v")

            wc1_sb = b64_sb[:, B64_WC1:B64_WC1 + 6 * CH]
            wc23_sb = b64_sb[:, B64_WC23:B64_WC23 + 6 * 128]
            ecl_sb = b64_sb[:, B64_ECL:B64_ECL + 66]
            ecr_sb = b64_sb[:, B64_ECR:B64_ECR + 67]
            ebl_sb = b64_sb[:, B64_EBL:B64_EBL + 34]
            ebr_sb = b64_sb[:, B64_EBR:B64_EBR + 35]
            w1_sb = b128_sb[:, B128_W1:B128_W1 + 6 * CH]
            w23_sb = b128_sb[:, B128_W23:B128_W23 + 6 * 128]
            b23_sb = b128_sb[:, B128_B23:B128_B23 + 1].bitcast(F32)
            b1_sb = b128_sb[0:CH, B128_B1:B128_B1 + 1].bitcast(F32)
            bv_sb = b128_sb[0:C, B128_BV:B128_BV + 1].bitcast(F32)

            # ---- input DMA head ----
            # sync/HWDGE: bundles + rows 0:64 of xb/xc (chunked);
            # gpsimd/SWDGE (Pool desc-gen): rows 64:128 + resid.
            # order: corr bundle, weights, xb head (Q0), then xc chunks
            # interleaved ahead of the remaining xb (Q1..3 run last)
            nc.sync.dma_start(out=b64_sb[:], in_=bund64.ap())
            nc.sync.dma_start(out=b128_sb[:], in_=bund128.ap())
            def _xb(lo, hi):
                nc.gpsimd.dma_start(out=xb_sb[C:128, lo:hi], in_=xb.ap()[C:128, lo:hi])
                nc.sync.dma_start(out=xb_sb[0:C, lo:hi], in_=xb.ap()[0:C, lo:hi])
            def _xc(lo, hi):
                nc.gpsimd.dma_start(out=xc_sb[C:128, lo:hi], in_=xc.ap()[C:128, lo:hi])
                nc.sync.dma_start(out=xc_sb[0:C, lo:hi], in_=xc.ap()[0:C, lo:hi])
            _xb(0, 704)
            _xc(0, 1152)
            _xc(1152, 2304)
            _xc(2304, 3456)
            _xc(3456, XC_LEN)
            _xb(704, 1408)
            _xb(1408, XB_LEN)

            # warmup scratch first: keeps the PE dummy transposes dep-free
            zwarm = big.tile([C, C], BF16, tag="zwarm")
            nc.vector.memset(zwarm[:], 0.0)
            # ones columns of V' (col 64 of each chunk) + exp table warm
            ones_bf = big.tile([128, NKC], BF16, tag="ones")
            nc.vector.memset(ones_bf[:], 1.0)
            v3 = v_sb[:].rearrange("p (a b) -> p a b", b=65)
            nc.vector.tensor_copy(v3[:, :, 64], ones_bf[:])
            warm = big.tile([128, 1], F32, tag="warm")
            nc.scalar.activation(warm[:], b23_sb[:], AF.Exp)
            identb = big.tile([C, C], BF16, tag="identb")
            make_identity(nc, identb[:])

            # ---- edge-correction matmuls (own PSUM bank, freed early) ----
            # layout (SBUF partition-aligned with consumers):
            #   Q-L [0:32, 0:32]   Q-R [0:32, 32:64]
            #   K-L [0:32, 64:128] K-R [0:32, 128:192]
            #   V-L [0:64, 192:256] V-R [0:64, 256:320]
            with tc.tile_pool(name="vtp", bufs=1, space="PSUM") as vtp:
                # PE pstate warmup: dummy transposes (no input deps) keep the
                # tensor engine continuously busy through the DMA head so the
                # 3us ramp to full clock completes before the real conv.
                pwarm = vtp.tile([C, C], F32, tag="pwarm")
                for _ in range(26):
                    nc.tensor.matmul(pwarm[:], zwarm[:], zwarm[:],
                                     start=True, stop=True)
                corrT = vtp.tile([C, 320], F32, tag="corrT")
                mms = []
                for dy in range(3):
                    # V first: the group's start zero-region must span the
                    # full partition range (0:64) of the tile.
                    mms.append((corrT[0:C, 192:256],
                                wc23_sb[:, (2 * dy) * 128 + 64:(2 * dy) * 128 + 128],
                                ecl_sb[:, dy:dy + 64]))
                    mms.append((corrT[0:C, 256:320],
                                wc23_sb[:, (2 * dy + 1) * 128 + 64:(2 * dy + 1) * 128 + 128],
                                ecr_sb[:, dy + 1:dy + 65]))
                    mms.append((corrT[0:CH, 0:32],
                                wc1_sb[:, (2 * dy) * CH:(2 * dy + 1) * CH],
                                ebl_sb[:, dy:dy + 32]))
                    mms.append((corrT[0:CH, 32:64],
                                wc1_sb[:, (2 * dy + 1) * CH:(2 * dy + 2) * CH],
                                ebr_sb[:, dy + 1:dy + 33]))
                    mms.append((corrT[0:CH, 64:128],
                                wc23_sb[:, (2 * dy) * 128:(2 * dy) * 128 + CH],
                                ecl_sb[:, dy:dy + 64]))
                    mms.append((corrT[0:CH, 128:192],
                                wc23_sb[:, (2 * dy + 1) * 128:(2 * dy + 1) * 128 + CH],
                                ecr_sb[:, dy + 1:dy + 65]))
                # first and last matmul must span the full 0:64 partition
                # range: start's zero region and stop's group close both
                # cover only the instruction's own partitions.
                mms = [mms[0]] + mms[2:] + [mms[1]]
                for i_mm, (o, l, r) in enumerate(mms):
                    nc.tensor.matmul(o, l, r,
                                     start=(i_mm == 0), stop=(i_mm == len(mms) - 1))
                nc.vector.tensor_copy(corrS[0:CH, 0:192], corrT[0:CH, 0:192])
                nc.vector.tensor_copy(corrS[0:C, 192:320], corrT[0:C, 192:320])
            corrQ = corrS[0:CH, 0:64]

            def conv_tile(pk, x_sb, w_sb, j, co):
                for dx in range(3):
                    base = j * 512 + dx
                    nc.tensor.matmul(
                        pk[:], w_sb[:, dx * co:(dx + 1) * co],
                        x_sb[:, base:base + 512],
                        start=(dx == 0), stop=False,
                    )
                for dx in range(3):
                    base = j * 512 + 2 * 64 + dx
                    nc.tensor.matmul(
                        pk[:], w_sb[0:C, (3 + dx) * co:(4 + dx) * co],
                        x_sb[0:C, base:base + 512],
                        start=False, stop=(dx == 2),
                    )

            # ---- conv + attention ----

            with tc.tile_pool(name="eps", bufs=2, space="PSUM") as eps:
                ets = {}

                def attn_qk(mt, p, pool, eng=None):
                    st = pool.tile([128, 1024], F32, tag="st")
                    for i in range(2):
                        kk = 2 * p + i
                        nc.tensor.matmul(
                            st[:, i * 512:(i + 1) * 512],
                            k_sb[:, kk * 128:(kk + 1) * 128],
                            qt_sb[:, mt * 512:(mt + 1) * 512],
                            start=True, stop=True,
                        )
                    ex = expool.tile([128, 1024], BF16, tag="ex")
                    if (eng or _exp_engine(mt, p)) == "act":
                        nc.scalar.activation(ex[:], st[:], AF.Exp, scale=1.0 / EXPA)
                    else:
                        nc.vector.tensor_scalar_add(ex[:].bitcast(I16), st[:], expb)
                    return ex

                def attn_pv(mt, p, ex):
                    for i in range(2):
                        kk = 2 * p + i
                        nc.tensor.matmul(
                            ets[mt][0:C + 1, :],
                            v_sb[:, kk * 65:kk * 65 + 65],
                            ex[:, i * 512:(i + 1) * 512],
                            start=(kk == 0), stop=(kk == NKC - 1),
                        )

                def attn_tail(mt):
                    et = ets[mt]
                    sls = [slice(h * 256, (h + 1) * 256) for h in range(2)]
                    osls = [slice(mt * 512 + h * 256, mt * 512 + (h + 1) * 256)
                            for h in range(2)]
                    bcs = []
                    for h in range(2):
                        recip = work.tile([1, 256], F32, tag="recip")
                        nc.vector.reciprocal(recip[:], et[C:C + 1, sls[h]])
                        bc = work.tile([C, 256], F32, tag="bc")
                        nc.gpsimd.partition_broadcast(bc[:], recip[:])
                        bcs.append(bc)
                    ots = []
                    for h in range(2):
                        ot = work.tile([C, 256], F32, tag="ot")
                        nc.vector.tensor_mul(ot[:], et[0:C, sls[h]], bcs[h][:])
                        ots.append(ot)
                    # h0's residual add on Pool, h1's on DVE: separate tiles
                    # and separate out-DMAs (two-writer single-tile DMA raced)
                    nc.gpsimd.tensor_add(ots[0][:], ots[0][:], res_sb[:, osls[0]])
                    nc.sync.dma_start(out=out_d.ap()[:, osls[0]], in_=ots[0][:])
                    nc.vector.tensor_add(ots[1][:], ots[1][:], res_sb[:, osls[1]])
                    nc.sync.dma_start(out=out_d.ap()[:, osls[1]], in_=ots[1][:])

                # ---- conv phase (attention PSUM not yet open) ----
                with (
                    tc.tile_pool(name="cps", bufs=2, space="PSUM") as cps,
                    tc.tile_pool(name="vtp2", bufs=2, space="PSUM") as vtp2,
                    tc.tile_pool(name="aps0", bufs=1, space="PSUM") as aps0,
                ):
                    def q_conv_tile(j):
                        pq0 = cps.tile([128, 512], F32, tag="cv")
                        pq = pq0[0:CH, :]
                        conv_tile(pq, xb_sb, w1_sb, j, CH)
                        qsl = qt_sb[:, j * 512:(j + 1) * 512]
                        nc.vector.tensor_scalar_add(qsl, pq[:], b1_sb)
                        q3 = qsl.rearrange("p (h w) -> p h w", w=64)
                        nc.vector.tensor_add(q3[:, :, 0], q3[:, :, 0], corrQ[:, j * 8:(j + 1) * 8])
                        nc.vector.tensor_add(q3[:, :, 63], q3[:, :, 63], corrQ[:, 32 + j * 8:32 + (j + 1) * 8])

                    def kv_conv_tile(j):
                        pk = cps.tile([128, 512], F32, tag="cv")
                        conv_tile(pk, xc_sb, w23_sb, j, 128)
                        ksl = k_sb[:, j * 512:(j + 1) * 512]
                        nc.vector.tensor_scalar_add(ksl, pk[0:CH, :], b23_sb[0:CH, :])
                        k3 = ksl.rearrange("p (h w) -> p h w", w=64)
                        nc.gpsimd.tensor_add(k3[:, :, 0], k3[:, :, 0], corrS[0:CH, 64 + j * 8:64 + (j + 1) * 8])
                        nc.gpsimd.tensor_add(k3[:, :, 63], k3[:, :, 63], corrS[0:CH, 128 + j * 8:128 + (j + 1) * 8])
                        vtmp = work.tile([C, 512], BF16, tag="vtmp")
                        nc.scalar.activation(vtmp[:], pk[64:128, :], AF.Identity, bias=bv_sb)
                        v3t = vtmp[:].rearrange("p (h w) -> p h w", w=64)
                        nc.vector.tensor_add(v3t[:, :, 0], v3t[:, :, 0], corrS[0:C, 192 + j * 8:192 + (j + 1) * 8])
                        nc.vector.tensor_add(v3t[:, :, 63], v3t[:, :, 63], corrS[0:C, 256 + j * 8:256 + (j + 1) * 8])
                        return vtmp

                    def v_transpose(j, vtmp):
                        vt = vtp2.tile([128, 256], BF16, tag="vt")
                        for c4 in range(4):
                            nc.tensor.transpose(
                                vt[:, c4 * 64:(c4 + 1) * 64],
                                vtmp[:, c4 * 128:(c4 + 1) * 128], identb[:])
                        vslc = v_sb[:, j * 260:(j + 1) * 260]
                        vdst = vslc.rearrange("p (c f) -> p c f", f=65)[:, :, 0:64]
                        nc.vector.tensor_copy(vdst, vt[:].rearrange("p (c f) -> p c f", f=64))

                    q_conv_tile(0)
                    et0 = eps.tile([128, 512], F32, tag="et")
                    ets[0] = et0
                    exs0 = {}
                    vtmps = {}
                    for j in range(8):
                        vtmps[j] = kv_conv_tile(j)
                        if j >= 2:
                            v_transpose(j - 2, vtmps.pop(j - 2))
                        # mt0 attention rides the conv's DMA-gated PE gaps:
                        # QK+exp of pair j-1, PV of pair j-2 (V ready then)
                        if j >= 1:
                            exs0[j - 1] = attn_qk(0, j - 1, aps0, eng="act")
                        if j >= 2:
                            attn_pv(0, j - 2, exs0.pop(j - 2))
                    exs0[7] = attn_qk(0, 7, aps0, eng="act")
                    v_transpose(6, vtmps.pop(6))
                    attn_pv(0, 6, exs0.pop(6))
                    q_conv_tile(1)
                    exs0[8] = attn_qk(0, 8, aps0, eng="act")
                    v_transpose(7, vtmps.pop(7))
                    attn_pv(0, 7, exs0.pop(7))
                    q_conv_tile(2)
                    exs0[9] = attn_qk(0, 9, aps0, eng="act")
                    q_conv_tile(3)
                    attn_pv(0, 8, exs0.pop(8))
                    nc.gpsimd.dma_start(out=res_sb[:], in_=resid.ap())

                if dbg:
                    nc.sync.dma_start(out=k_d.ap(), in_=k_sb[:].bitcast(F32))
                    nc.sync.dma_start(out=q_d.ap(), in_=qt_sb[:].bitcast(F32))
                    vdbg = big.tile([128, NKC * 65], F32, tag="vdbg")
                    nc.vector.tensor_copy(vdbg[:], v_sb[:])
                    nc.sync.dma_start(out=v_d.ap(), in_=vdbg[:])

                # ---- attention phase: all 4 mts, three-stage pipeline ----
                with tc.tile_pool(name="aps2", bufs=3, space="PSUM") as aps2:
                    for mt in range(4):
                        if mt > 0:
                            etm = eps.tile([128, 512], F32, tag="et")
                            ets[mt] = etm
                        pend = [(9, exs0.pop(9))] if mt == 0 else []
                        for p in range(10 if mt == 0 else 0, NPAIR):
                            pend.append((p, attn_qk(mt, p, aps2)))
                            if p == 1 and mt > 0:
                                attn_tail(mt - 1)
                            if len(pend) > 2:
                                pp, pex = pend.pop(0)
                                attn_pv(mt, pp, pex)
                        for pp, pex in pend:
                            attn_pv(mt, pp, pex)
                    attn_tail(3)

    nc.compile()
    return nc


def _prep_core_inputs(inputs, core):
    A1_B = np.asarray(inputs["A1_B"], np.float32)
    A1_C = np.asarray(inputs["A1_C"], np.float32)
    w1 = np.asarray(inputs["w1"], np.float32) * EXPA
    b1 = np.asarray(inputs["b1"], np.float32) * EXPA
    w2 = np.asarray(inputs["w2"], np.float32)
    b2 = np.asarray(inputs["b2"], np.float32)
    w3 = np.asarray(inputs["w3"], np.float32)
    b3 = np.asarray(inputs["b3"], np.float32)
    b = core // 2
    half = core % 2
    h0 = half * 32

    xc = np.zeros((128, XC_LEN), np.float32)
    flat_c = np.zeros((C, H + 2, W), np.float32)
    flat_c[:, 1:H + 1, :] = A1_C[b]
    xc[0:C, 1:1 + (H + 2) * W] = flat_c.reshape(C, -1)
    xc[C:128, 0:XC_LEN - 64] = xc[0:C, 64:XC_LEN]

    xb = np.zeros((128, XB_LEN), np.float32)
    flat_b = np.zeros((C, 34, W), np.float32)
    glo = h0 - 1
    src_lo = max(glo, 0)
    src_hi = min(h0 + 33, H)
    flat_b[:, src_lo - glo: src_hi - glo, :] = A1_B[b][:, src_lo:src_hi, :]
    xb[0:C, 1:1 + 34 * W] = flat_b.reshape(C, -1)
    xb[C:128, 0:XB_LEN - 64] = xb[0:C, 64:XB_LEN]

    # Tap layout: cols t*co.. with t=0..2 the (dy=0,dy=1) stacked pairs
    # (contraction rows 0:64 = dy0, 64:128 = dy1), t=3..5 the dy=2 singles.
    w1t = np.zeros((128, 6 * CH), np.float32)
    w23t = np.zeros((128, 6 * 128), np.float32)
    for dx in range(3):
        w1t[0:C, dx * CH:(dx + 1) * CH] = w1[:, :, 0, dx].T
        w1t[C:128, dx * CH:(dx + 1) * CH] = w1[:, :, 1, dx].T
        w1t[0:C, (3 + dx) * CH:(4 + dx) * CH] = w1[:, :, 2, dx].T
        w23t[0:C, dx * 128: dx * 128 + CH] = w2[:, :, 0, dx].T
        w23t[C:128, dx * 128: dx * 128 + CH] = w2[:, :, 1, dx].T
        w23t[0:C, dx * 128 + 64: dx * 128 + 128] = w3[:, :, 0, dx].T
        w23t[C:128, dx * 128 + 64: dx * 128 + 128] = w3[:, :, 1, dx].T
        w23t[0:C, (3 + dx) * 128: (3 + dx) * 128 + CH] = w2[:, :, 2, dx].T
        w23t[0:C, (3 + dx) * 128 + 64: (3 + dx) * 128 + 128] = w3[:, :, 2, dx].T
    wc1 = np.zeros((C, 6 * CH), np.float32)
    wc23 = np.zeros((C, 6 * 128), np.float32)
    for dy in range(3):
        for side, dx in ((0, 0), (1, 2)):
            i = 2 * dy + side
            wc1[:, i * CH:(i + 1) * CH] = -w1[:, :, dy, dx].T
            wc23[:, i * 128: i * 128 + CH] = -w2[:, :, dy, dx].T
            wc23[:, i * 128 + 64: i * 128 + 128] = -w3[:, :, dy, dx].T

    xcr = _r32r(xc)
    xbr = _r32r(xb)

    bund64 = np.zeros((C, B64_LEN), np.float32)  # cast to bf16 at the end
    bund64[:, B64_WC1:B64_WC1 + 6 * CH] = _r32r(wc1)
    bund64[:, B64_WC23:B64_WC23 + 6 * 128] = _r32r(wc23)
    bund64[:, B64_ECL:B64_ECL + 66] = xcr[0:C, (np.arange(66)) * 64]
    bund64[:, B64_ECR:B64_ECR + 67] = xcr[0:C, (np.arange(67)) * 64 + 1]
    bund64[:, B64_EBL:B64_EBL + 34] = xbr[0:C, (np.arange(34)) * 64]
    bund64[:, B64_EBR:B64_EBR + 35] = xbr[0:C, (np.arange(35)) * 64 + 1]

    bund128 = np.zeros((128, B128_LEN), np.float32)
    bund128[:, B128_W1:B128_W1 + 6 * CH] = _r32r(w1t)
    bund128[:, B128_W23:B128_W23 + 6 * 128] = _r32r(w23t)
    bund128[:, B128_B23] = np.concatenate([b2, np.zeros(32, np.float32), b3])
    bund128[0:CH, B128_B1] = b1
    bund128[0:C, B128_BV] = b3

    resid = np.ascontiguousarray(A1_C[b][:, h0:h0 + 32, :].reshape(C, M))
    import ml_dtypes
    bund64 = bund64.astype(ml_dtypes.bfloat16)
    return {
        "xc": xcr,
        "xb": xbr,
        "bund64": bund64,
        "bund128": bund128,
        "resid": resid,
    }


def _run(inputs, trace=False, dbg=False):
    key = ("nc", dbg)
    if key not in _cache:
        _cache[key] = _build(dbg)
    nc = _cache[key]
    in_maps = [_prep_core_inputs(inputs, i) for i in range(NCORES)]
    res = bass_utils.run_bass_kernel_spmd(
        nc, in_maps, core_ids=list(range(NCORES)), trace=trace)
    out = np.empty((B, C, H, W), np.float32)
    for i in range(NCORES):
        b, half = i // 2, i % 2
        out[b, :, half * 32:half * 32 + 32, :] = res.results[i]["out"].reshape(C, 32, W)
    return out, res


def kernel(**inputs):
    out, _ = _run(inputs, trace=False)
    return out
